# revision 4
# baseline (speedup 1.0000x reference)
"""Batch beam-search step kernel for Trainium2 (8 NeuronCores, batch-sharded).

Reference semantics (per batch b, beams K=8, vocab V=50257):
  logprobs = log_softmax(logits); new = logprobs + cand_scores[:, None]
  eos = new[:, :, EOS]; completed top-k over concat(completed, eos)/len
  uncompleted top-8 over new with EOS masked, gather seqs + decoder state.

Sharding: batch dim 128 -> 16 batches per core (128 (b,k) rows = 128
SBUF partitions per core).

Device algorithm per core:
  Phase A (streaming, memory-bound): logits padded to 99*512 columns,
  streamed in 9 chunks of 11*512; per chunk: 3D reduce_max -> per-cell
  (512-wide) maxes, then in-place Exp with accum -> per-chunk sum(exp).
  Phase B: logsumexp from chunk sums; top-8 cells per row via max/max_index
  over the 99 cell-maxes; one indirect DMA gathers the 8 winning cells
  (8*512 f32) per row; max/max_index over the gathered 4K recovers the
  row-global top-8 values and vocab indices.  Per-batch top-8 over the
  8x8 row candidates via a DRAM bounce to batch-major layout; beam
  reorder of seqs/decoder state via indirect row gathers by parent.
"""

import numpy as np

B, K, V = 128, 8, 50257
L, MAXL = 64, 128
A, D = 512, 1024
EOS = 2
NEG = -1.0e9

NCORES = 8
BPC = B // NCORES          # 16 batches per core
ROWS = BPC * K             # 128 rows = partitions
CW = 512                   # cell width
NCELLS = 99                # ceil(V / CW)
VP = NCELLS * CW           # 50688 padded vocab
CC = 11                    # cells per chunk
NCHUNK = NCELLS // CC      # 9 chunks
CHUNK = CC * CW            # 5632

_f32 = np.float32
_i32 = np.int32


def build_nc():
    import concourse.bacc as bacc
    import concourse.mybir as mybir
    from concourse.bass import IndirectOffsetOnAxis
    from concourse.tile import TileContext

    dt = mybir.dt
    AX = mybir.AxisListType
    OP = mybir.AluOpType
    AF = mybir.ActivationFunctionType

    nc = bacc.Bacc("TRN2", target_bir_lowering=False, num_devices=NCORES)

    # ---- inputs ----
    lg = nc.declare_dram_parameter("lg", [ROWS * NCELLS, CW], dt.float32, isOutput=False)
    eosl = nc.declare_dram_parameter("eosl", [ROWS, 1], dt.float32, isOutput=False)
    csc = nc.declare_dram_parameter("csc", [ROWS, 1], dt.float32, isOutput=False)
    cseqt = nc.declare_dram_parameter("cseqt", [ROWS, L], dt.int32, isOutput=False)
    compsc = nc.declare_dram_parameter("compsc", [BPC, K], dt.float32, isOutput=False)
    complenf = nc.declare_dram_parameter("complenf", [BPC, K], dt.float32, isOutput=False)
    comptab = nc.declare_dram_parameter("comptab", [BPC * 2 * K, MAXL], dt.int32, isOutput=False)
    dctx = nc.declare_dram_parameter("dctx", [ROWS, A], dt.float32, isOutput=False)
    dr1 = nc.declare_dram_parameter("dr1", [ROWS, D], dt.float32, isOutput=False)
    dr2 = nc.declare_dram_parameter("dr2", [ROWS, D], dt.float32, isOutput=False)
    iota64 = nc.declare_dram_parameter("iota64", [BPC, K * K], dt.float32, isOutput=False)
    iota8 = nc.declare_dram_parameter("iota8", [ROWS, K], dt.float32, isOutput=False)
    rb99 = nc.declare_dram_parameter("rb99", [ROWS, 1], dt.int32, isOutput=False)
    rb8 = nc.declare_dram_parameter("rb8", [ROWS, 1], dt.int32, isOutput=False)
    rb16 = nc.declare_dram_parameter("rb16", [ROWS, 1], dt.int32, isOutput=False)

    # ---- outputs ----
    ncs_o = nc.declare_dram_parameter("ncs", [BPC, K], dt.float32, isOutput=True)
    nseq_o = nc.declare_dram_parameter("nseq", [ROWS, L + 1], dt.int32, isOutput=True)
    par_o = nc.declare_dram_parameter("par", [BPC, K], dt.int32, isOutput=True)
    cso_o = nc.declare_dram_parameter("cso", [BPC, K], dt.float32, isOutput=True)
    cseq_o = nc.declare_dram_parameter("cseqo", [ROWS, MAXL], dt.int32, isOutput=True)
    clen_o = nc.declare_dram_parameter("cleno", [BPC, K], dt.int32, isOutput=True)
    ctx_o = nc.declare_dram_parameter("ctxo", [ROWS, A], dt.float32, isOutput=True)
    r1_o = nc.declare_dram_parameter("r1o", [ROWS, D], dt.float32, isOutput=True)
    r2_o = nc.declare_dram_parameter("r2o", [ROWS, D], dt.float32, isOutput=True)

    # ---- DRAM bounce scratch ----
    sc_d = nc.dram_tensor("sc_d", [ROWS, K], dt.float32)
    sym_d = nc.dram_tensor("sym_d", [ROWS, K], dt.float32)
    par_d = nc.dram_tensor("par_d", [BPC, K], dt.int32)
    symsel_d = nc.dram_tensor("symsel_d", [BPC, K], dt.float32)
    es_d = nc.dram_tensor("es_d", [ROWS, 1], dt.float32)
    cind_d = nc.dram_tensor("cind_d", [BPC, K], dt.int32)

    lgv = lg.ap().rearrange("(p n) w -> p (n w)", p=ROWS)

    with TileContext(nc) as tc:
        with (
            tc.tile_pool(name="stream", bufs=4) as stream,
            tc.tile_pool(name="persist", bufs=1) as persist,
            tc.tile_pool(name="small", bufs=1) as small,
        ):
            cellmax = persist.tile([ROWS, NCELLS], dt.float32)
            scol = persist.tile([ROWS, NCHUNK], dt.float32)

            # ---------- Phase A: stream logits ----------
            for c in range(NCHUNK):
                t = stream.tile([ROWS, CHUNK], dt.float32, tag="chunk")
                nc.sync.dma_start(out=t[:], in_=lgv[:, c * CHUNK:(c + 1) * CHUNK])
                nc.vector.reduce_max(
                    cellmax[:, c * CC:(c + 1) * CC],
                    t.rearrange("p (n w) -> p n w", w=CW),
                    axis=AX.X,
                )
                nc.scalar.activation(t[:], t[:], AF.Exp, accum_out=scol[:, c:c + 1])

            # ---------- Phase B ----------
            # logsumexp (EOS column was masked to NEG on host; add exp back)
            eosl_sb = small.tile([ROWS, 1], dt.float32)
            nc.sync.dma_start(out=eosl_sb[:], in_=eosl[:])
            csc_sb = small.tile([ROWS, 1], dt.float32)
            nc.sync.dma_start(out=csc_sb[:], in_=csc[:])

            S = small.tile([ROWS, 1], dt.float32)
            nc.vector.reduce_sum(S[:], scol[:], axis=AX.X)
            eexp = small.tile([ROWS, 1], dt.float32)
            nc.scalar.activation(eexp[:], eosl_sb[:], AF.Exp)
            S2 = small.tile([ROWS, 1], dt.float32)
            nc.vector.tensor_add(S2[:], S[:], eexp[:])
            lse = small.tile([ROWS, 1], dt.float32)
            nc.scalar.activation(lse[:], S2[:], AF.Ln)
            rc = small.tile([ROWS, 1], dt.float32)
            nc.vector.tensor_sub(rc[:], csc_sb[:], lse[:])
            eos_adj = small.tile([ROWS, 1], dt.float32)
            nc.vector.tensor_add(eos_adj[:], eosl_sb[:], rc[:])

            # top-8 cells per row
            c8v = small.tile([ROWS, K], dt.float32)
            nc.vector.max(c8v[:], cellmax[:])
            c8i = small.tile([ROWS, K], dt.uint32)
            nc.vector.max_index(c8i[:], c8v[:], cellmax[:])
            c8ii = small.tile([ROWS, K], dt.int32)
            nc.vector.tensor_copy(c8ii[:], c8i[:])
            rb99_sb = small.tile([ROWS, 1], dt.int32)
            nc.sync.dma_start(out=rb99_sb[:], in_=rb99[:])
            rowid = small.tile([ROWS, K], dt.int32)
            nc.vector.tensor_tensor(
                out=rowid[:], in0=c8ii[:], in1=rb99_sb.to_broadcast([ROWS, K]),
                op=OP.add,
            )

            # gather the 8 winning cells per row
            # HW indirect DMA uses one offset per partition (contiguous read of
            # out-free-size elements) — issue one gather per winner slot j.
            G = persist.tile([ROWS, K * CW], dt.float32)
            for j in range(K):
                nc.gpsimd.indirect_dma_start(
                    out=G[:, j * CW:(j + 1) * CW],
                    out_offset=None,
                    in_=lg.ap(),
                    in_offset=IndirectOffsetOnAxis(ap=rowid[:, j:j + 1], axis=0),
                )
            g8v = small.tile([ROWS, K], dt.float32)
            nc.vector.max(g8v[:], G[:])
            g8i = small.tile([ROWS, K], dt.uint32)
            nc.vector.max_index(g8i[:], g8v[:], G[:])

            jstar = small.tile([ROWS, K], dt.uint32)
            nc.vector.tensor_scalar(
                out=jstar[:], in0=g8i[:], scalar1=9, scalar2=None,
                op0=OP.logical_shift_right,
            )
            within = small.tile([ROWS, K], dt.uint32)
            nc.vector.tensor_scalar(
                out=within[:], in0=g8i[:], scalar1=511, scalar2=None,
                op0=OP.bitwise_and,
            )
            jstar_f = small.tile([ROWS, K], dt.float32)
            nc.vector.tensor_copy(jstar_f[:], jstar[:])
            c8f = small.tile([ROWS, K], dt.float32)
            nc.vector.tensor_copy(c8f[:], c8ii[:])
            withf = small.tile([ROWS, K], dt.float32)
            nc.vector.tensor_copy(withf[:], within[:])

            iota8_sb = small.tile([ROWS, K], dt.float32)
            nc.sync.dma_start(out=iota8_sb[:], in_=iota8[:])

            # cellsel[p, j] = c8f[p, jstar[p, j]]  (batched one-hot gather)
            mask88 = small.tile([ROWS, K, K], dt.float32)
            nc.vector.tensor_tensor(
                out=mask88[:],
                in0=iota8_sb.unsqueeze(1).to_broadcast([ROWS, K, K]),
                in1=jstar_f.unsqueeze(2).to_broadcast([ROWS, K, K]),
                op=OP.is_equal,
            )
            prod88 = small.tile([ROWS, K, K], dt.float32)
            nc.vector.tensor_tensor(
                out=prod88[:],
                in0=mask88[:],
                in1=c8f.unsqueeze(1).to_broadcast([ROWS, K, K]),
                op=OP.mult,
            )
            cellsel = small.tile([ROWS, K], dt.float32)
            nc.vector.reduce_sum(cellsel[:], prod88[:], axis=AX.X)

            voc = small.tile([ROWS, K], dt.float32)
            nc.vector.tensor_scalar(
                out=voc[:], in0=cellsel[:], scalar1=float(CW), scalar2=None,
                op0=OP.mult,
            )
            nc.vector.tensor_add(voc[:], voc[:], withf[:])

            g8adj = small.tile([ROWS, K], dt.float32)
            nc.vector.tensor_scalar(
                out=g8adj[:], in0=g8v[:], scalar1=rc[:, 0:1], scalar2=None,
                op0=OP.add,
            )

            # bounce row candidates to batch-major [16, 64]
            nc.sync.dma_start(out=sc_d.ap(), in_=g8adj[:])
            nc.sync.dma_start(out=sym_d.ap(), in_=voc[:])
            b64v = small.tile([BPC, K * K], dt.float32)
            nc.sync.dma_start(out=b64v[:], in_=sc_d.ap().rearrange("(b k) j -> b (k j)", k=K))
            b64s = small.tile([BPC, K * K], dt.float32)
            nc.sync.dma_start(out=b64s[:], in_=sym_d.ap().rearrange("(b k) j -> b (k j)", k=K))

            ncs8 = small.tile([BPC, K], dt.float32)
            nc.vector.max(ncs8[:], b64v[:])
            nc.sync.dma_start(out=ncs_o.ap(), in_=ncs8[:])
            pos = small.tile([BPC, K], dt.uint32)
            nc.vector.max_index(pos[:], ncs8[:], b64v[:])
            posf = small.tile([BPC, K], dt.float32)
            nc.vector.tensor_copy(posf[:], pos[:])
            par_u = small.tile([BPC, K], dt.uint32)
            nc.vector.tensor_scalar(
                out=par_u[:], in0=pos[:], scalar1=3, scalar2=None,
                op0=OP.logical_shift_right,
            )
            par_i = small.tile([BPC, K], dt.int32)
            nc.vector.tensor_copy(par_i[:], par_u[:])
            nc.sync.dma_start(out=par_o.ap(), in_=par_i[:])
            nc.sync.dma_start(out=par_d.ap(), in_=par_i[:])

            iota64_sb = small.tile([BPC, K * K], dt.float32)
            nc.sync.dma_start(out=iota64_sb[:], in_=iota64[:])

            # symsel[b, j] = b64s[b, pos[b, j]]
            KK = K * K
            m64 = small.tile([BPC, K, KK], dt.float32)
            nc.vector.tensor_tensor(
                out=m64[:],
                in0=iota64_sb.unsqueeze(1).to_broadcast([BPC, K, KK]),
                in1=posf.unsqueeze(2).to_broadcast([BPC, K, KK]),
                op=OP.is_equal,
            )
            p64 = small.tile([BPC, K, KK], dt.float32)
            nc.vector.tensor_tensor(
                out=p64[:], in0=m64[:],
                in1=b64s.unsqueeze(1).to_broadcast([BPC, K, KK]),
                op=OP.mult,
            )
            symsel = small.tile([BPC, K], dt.float32)
            nc.vector.reduce_sum(symsel[:], p64[:], axis=AX.X)
            nc.sync.dma_start(out=symsel_d.ap(), in_=symsel[:])

            # parent row gathers
            par128 = small.tile([ROWS, 1], dt.int32)
            nc.sync.dma_start(out=par128[:], in_=par_d.ap().rearrange("b j -> (b j) ()"))
            sym128f = small.tile([ROWS, 1], dt.float32)
            nc.sync.dma_start(out=sym128f[:], in_=symsel_d.ap().rearrange("b j -> (b j) ()"))
            sym128i = small.tile([ROWS, 1], dt.int32)
            nc.vector.tensor_copy(sym128i[:], sym128f[:])
            rb8_sb = small.tile([ROWS, 1], dt.int32)
            nc.sync.dma_start(out=rb8_sb[:], in_=rb8[:])
            prow = small.tile([ROWS, 1], dt.int32)
            nc.vector.tensor_add(prow[:], par128[:], rb8_sb[:])

            ctxg = persist.tile([ROWS, A], dt.float32)
            nc.gpsimd.indirect_dma_start(
                out=ctxg[:], out_offset=None, in_=dctx.ap(),
                in_offset=IndirectOffsetOnAxis(ap=prow[:], axis=0),
            )
            nc.sync.dma_start(out=ctx_o.ap(), in_=ctxg[:])
            r1g = persist.tile([ROWS, D], dt.float32)
            nc.gpsimd.indirect_dma_start(
                out=r1g[:], out_offset=None, in_=dr1.ap(),
                in_offset=IndirectOffsetOnAxis(ap=prow[:], axis=0),
            )
            nc.sync.dma_start(out=r1_o.ap(), in_=r1g[:])
            r2g = persist.tile([ROWS, D], dt.float32)
            nc.gpsimd.indirect_dma_start(
                out=r2g[:], out_offset=None, in_=dr2.ap(),
                in_offset=IndirectOffsetOnAxis(ap=prow[:], axis=0),
            )
            nc.sync.dma_start(out=r2_o.ap(), in_=r2g[:])
            seqg = persist.tile([ROWS, L], dt.int32)
            nc.gpsimd.indirect_dma_start(
                out=seqg[:], out_offset=None, in_=cseqt.ap(),
                in_offset=IndirectOffsetOnAxis(ap=prow[:], axis=0),
            )
            nc.sync.dma_start(out=nseq_o.ap()[:, 0:L], in_=seqg[:])
            nc.sync.dma_start(out=nseq_o.ap()[:, L:L + 1], in_=sym128i[:])

            # ---------- completed path ----------
            nc.sync.dma_start(out=es_d.ap(), in_=eos_adj[:])
            comb = small.tile([BPC, 2 * K], dt.float32)
            nc.sync.dma_start(out=comb[:, 0:K], in_=compsc.ap())
            nc.sync.dma_start(out=comb[:, K:2 * K], in_=es_d.ap().rearrange("(b k) o -> b (k o)", k=K))
            clen = small.tile([BPC, 2 * K], dt.float32)
            nc.sync.dma_start(out=clen[:, 0:K], in_=complenf.ap())
            nc.vector.memset(clen[:, K:2 * K], float(L + 1))
            rin = small.tile([BPC, 2 * K], dt.float32)
            nc.vector.reciprocal(rin[:], clen[:])
            resc = small.tile([BPC, 2 * K], dt.float32)
            nc.vector.tensor_mul(resc[:], comb[:], rin[:])
            cc8 = small.tile([BPC, K], dt.float32)
            nc.vector.max(cc8[:], resc[:])
            cind = small.tile([BPC, K], dt.uint32)
            nc.vector.max_index(cind[:], cc8[:], resc[:])
            cindf = small.tile([BPC, K], dt.float32)
            nc.vector.tensor_copy(cindf[:], cind[:])

            K2 = 2 * K
            m16 = small.tile([BPC, K, K2], dt.float32)
            nc.vector.tensor_tensor(
                out=m16[:],
                in0=iota64_sb[:, 0:K2].unsqueeze(1).to_broadcast([BPC, K, K2]),
                in1=cindf.unsqueeze(2).to_broadcast([BPC, K, K2]),
                op=OP.is_equal,
            )
            p16 = small.tile([BPC, K, K2], dt.float32)
            nc.vector.tensor_tensor(
                out=p16[:], in0=m16[:],
                in1=comb.unsqueeze(1).to_broadcast([BPC, K, K2]),
                op=OP.mult,
            )
            cso8 = small.tile([BPC, K], dt.float32)
            nc.vector.reduce_sum(cso8[:], p16[:], axis=AX.X)
            nc.sync.dma_start(out=cso_o.ap(), in_=cso8[:])
            p16b = small.tile([BPC, K, K2], dt.float32)
            nc.vector.tensor_tensor(
                out=p16b[:], in0=m16[:],
                in1=clen.unsqueeze(1).to_broadcast([BPC, K, K2]),
                op=OP.mult,
            )
            clenf8 = small.tile([BPC, K], dt.float32)
            nc.vector.reduce_sum(clenf8[:], p16b[:], axis=AX.X)
            cleni = small.tile([BPC, K], dt.int32)
            nc.vector.tensor_copy(cleni[:], clenf8[:])
            nc.sync.dma_start(out=clen_o.ap(), in_=cleni[:])

            cindi = small.tile([BPC, K], dt.int32)
            nc.vector.tensor_copy(cindi[:], cind[:])
            nc.sync.dma_start(out=cind_d.ap(), in_=cindi[:])
            crow = small.tile([ROWS, 1], dt.int32)
            nc.sync.dma_start(out=crow[:], in_=cind_d.ap().rearrange("b j -> (b j) ()"))
            rb16_sb = small.tile([ROWS, 1], dt.int32)
            nc.sync.dma_start(out=rb16_sb[:], in_=rb16[:])
            crow2 = small.tile([ROWS, 1], dt.int32)
            nc.vector.tensor_add(crow2[:], crow[:], rb16_sb[:])
            cseqg = persist.tile([ROWS, MAXL], dt.int32)
            nc.gpsimd.indirect_dma_start(
                out=cseqg[:], out_offset=None, in_=comptab.ap(),
                in_offset=IndirectOffsetOnAxis(ap=crow2[:], axis=0),
            )
            nc.sync.dma_start(out=cseq_o.ap(), in_=cseqg[:])

    nc.compile()
    return nc


# ---------------------------------------------------------------------------
# host side
# ---------------------------------------------------------------------------

_NC_CACHE = None


def _get_nc():
    global _NC_CACHE
    if _NC_CACHE is None:
        _NC_CACHE = build_nc()
    return _NC_CACHE


def make_in_maps(logits, cand_scores, cand_seqs, completed_scores, completed_seqs,
                 completed_length, decoder_context, decoder_rnn1, decoder_rnn2):
    """Shard + preprocess full inputs into per-core input maps."""
    logits = np.asarray(logits, dtype=_f32)
    cand_scores = np.asarray(cand_scores, dtype=_f32)
    cand_seqs = np.asarray(cand_seqs, dtype=_i32)
    completed_scores = np.asarray(completed_scores, dtype=_f32)
    completed_seqs = np.asarray(completed_seqs, dtype=_i32)
    completed_length = np.asarray(completed_length, dtype=_i32)
    decoder_context = np.asarray(decoder_context, dtype=_f32).reshape(B, K, A)
    decoder_rnn1 = np.asarray(decoder_rnn1, dtype=_f32).reshape(B, K, D)
    decoder_rnn2 = np.asarray(decoder_rnn2, dtype=_f32).reshape(B, K, D)

    p = np.arange(ROWS, dtype=_i32)[:, None]
    iota8 = np.broadcast_to(np.arange(K, dtype=_f32), (ROWS, K)).copy()
    iota64 = np.broadcast_to(np.arange(K * K, dtype=_f32), (BPC, K * K)).copy()
    rb99 = (p * NCELLS).astype(_i32)
    rb8 = ((p // K) * K).astype(_i32)
    rb16 = ((p // K) * 2 * K).astype(_i32)

    in_maps = []
    for c in range(NCORES):
        b0, b1 = c * BPC, (c + 1) * BPC
        lg2 = logits[b0:b1].reshape(ROWS, V)
        eosl = lg2[:, EOS:EOS + 1].copy()
        lgp = np.full((ROWS, VP), NEG, dtype=_f32)
        lgp[:, :V] = lg2
        lgp[:, EOS] = NEG

        # completed table rows (b*16 + c): c<8 completed_seqs, c>=8 cand_seqs+EOS
        ctab = np.full((BPC, 2 * K, MAXL), EOS, dtype=_i32)
        ctab[:, :K, :] = completed_seqs[b0:b1]
        ctab[:, K:, :L] = cand_seqs[b0:b1]

        in_maps.append({
            "lg": lgp.reshape(ROWS * NCELLS, CW),
            "eosl": eosl,
            "csc": cand_scores[b0:b1].reshape(ROWS, 1),
            "cseqt": cand_seqs[b0:b1].reshape(ROWS, L),
            "compsc": completed_scores[b0:b1],
            "complenf": completed_length[b0:b1].astype(_f32),
            "comptab": ctab.reshape(BPC * 2 * K, MAXL),
            "dctx": decoder_context[b0:b1].reshape(ROWS, A),
            "dr1": decoder_rnn1[b0:b1].reshape(ROWS, D),
            "dr2": decoder_rnn2[b0:b1].reshape(ROWS, D),
            "iota64": iota64,
            "iota8": iota8,
            "rb99": rb99,
            "rb8": rb8,
            "rb16": rb16,
        })
    return in_maps


def assemble_outputs(results):
    """Concatenate per-core output dicts into the reference's 9-tuple."""
    cat = lambda name: np.concatenate([np.asarray(r[name]) for r in results], axis=0)
    new_cand_scores = cat("ncs").reshape(B, K).astype(_f32)
    new_seqs = cat("nseq").reshape(B, K, L + 1).astype(_i32)
    parents = cat("par").reshape(B, K).astype(_i32)
    comp_scores = cat("cso").reshape(B, K).astype(_f32)
    comp_seqs = cat("cseqo").reshape(B, K, MAXL).astype(_i32)
    comp_len = cat("cleno").reshape(B, K).astype(_i32)
    ctx = cat("ctxo").reshape(B * K, A).astype(_f32)
    r1 = cat("r1o").reshape(B * K, D).astype(_f32)
    r2 = cat("r2o").reshape(B * K, D).astype(_f32)
    return (new_cand_scores, new_seqs, parents, comp_scores, comp_seqs,
            comp_len, ctx, r1, r2)


def kernel(**inputs):
    from concourse.bass_utils import run_bass_kernel_spmd
    nc = _get_nc()
    in_maps = make_in_maps(**inputs)
    res = run_bass_kernel_spmd(nc, in_maps, list(range(NCORES)))
    return assemble_outputs(res.results)


# revision 34
# speedup vs baseline: 1.2068x; 1.2068x over previous
"""Batch beam-search step kernel for Trainium2 (8 NeuronCores, batch-sharded).

Reference semantics (per batch b, beams K=8, vocab V=50257):
  logprobs = log_softmax(logits); new = logprobs + cand_scores[:, None]
  eos = new[:, :, EOS]; completed top-k over concat(completed, eos)/len
  uncompleted top-8 over new with EOS masked, gather seqs + decoder state.

Sharding: batch dim 128 -> 16 batches per core (128 (b,k) rows = 128
SBUF partitions per core).

Device algorithm per core:
  Phase A (streaming, memory-bound): logits padded to 393*128 columns,
  streamed in tapered chunks; per chunk a 3D reduce_max produces per-cell
  (128-wide) maxes while Exp(+accum) computes the softmax denominator in
  parallel on the scalar engine.
  Phase B: logsumexp from chunk sums; top-8 cells per row via max/max_index
  over the 393 cell maxes; 8 indirect DMAs gather the winning cells;
  max/max_index over the gathered 1K recovers the row-global top-8 values
  and vocab ids.  Per-batch top-8 over the 8x8 row candidates via a DRAM
  bounce to batch-major layout; beam reorder of seqs + decoder state via
  one indirect row gather by parent over a host-packed [ctx|r1|r2|seq]
  table.
"""

import numpy as np

B, K, V = 128, 8, 50257
L, MAXL = 64, 128
A, D = 512, 1024
EOS = 2
NEG = -1.0e9

NCORES = 8
BPC = B // NCORES          # 16 batches per core
ROWS = BPC * K             # 128 rows = partitions
CW = 128                   # cell width
NCELLS = 393               # ceil(V / CW)
VP = NCELLS * CW           # 50304 padded vocab
# cells per streaming chunk (tapered tail so the last reduce+exp are short)
CHUNK_CELLS = [45, 45, 45, 45, 45, 45, 45, 36, 21, 12, 6, 3]
assert sum(CHUNK_CELLS) == NCELLS
NCHUNK = len(CHUNK_CELLS)
DS = A + 2 * D + L         # packed decoder-state row: ctx|r1|r2|seq = 2624

_f32 = np.float32
_i32 = np.int32


def build_nc(repeat=1):
    import contextlib
    import concourse.bacc as bacc
    import concourse.mybir as mybir
    from concourse.bass import IndirectOffsetOnAxis
    from concourse.tile import TileContext

    dt = mybir.dt
    AX = mybir.AxisListType
    OP = mybir.AluOpType
    AF = mybir.ActivationFunctionType

    nc = bacc.Bacc("TRN2", target_bir_lowering=False, num_devices=NCORES)

    # ---- inputs ----
    lg = nc.declare_dram_parameter("lg", [ROWS * NCELLS, CW], dt.float32, isOutput=False)
    # c128: iota8(0:8)|eosl(8)|csc(9)|rb393(10,i32)|rb8(11,i32)|rb16(12,i32)|rb8f(13)
    c128 = nc.declare_dram_parameter("c128", [ROWS, 14], dt.float32, isOutput=False)
    # c16: iota64(0:64) | compsc(64:72) | complenf(72:80) | bsel(80:208)
    c16 = nc.declare_dram_parameter("c16", [BPC, 208], dt.float32, isOutput=False)
    # pec: lgsel selectors (0:128; selector k at cols 16k..16k+16) | diag8 (128:136)
    pec = nc.declare_dram_parameter("pec", [ROWS, 136], dt.float32, isOutput=False)
    comptab = nc.declare_dram_parameter("comptab", [BPC * 2 * K, MAXL], dt.int32, isOutput=False)
    dstate = nc.declare_dram_parameter("dstate", [ROWS, DS], dt.float32, isOutput=False)

    # ---- outputs ----
    # out40 packs [ncs | parents | comp_scores | comp_len | syms] per batch (f32).
    out40_o = nc.declare_dram_parameter("out40", [BPC, 5 * K], dt.float32, isOutput=True)
    cseq_o = nc.declare_dram_parameter("cseqo", [ROWS, MAXL], dt.int32, isOutput=True)
    dso_o = nc.declare_dram_parameter("dso", [ROWS, DS], dt.float32, isOutput=True)

    # ---- DRAM bounce scratch ----
    par_d = nc.dram_tensor("par_d", [BPC, K], dt.int32)
    es_d = nc.dram_tensor("es_d", [ROWS, 1], dt.float32)
    cind_d = nc.dram_tensor("cind_d", [BPC, K], dt.int32)

    lgv = lg.ap().rearrange("(p n) w -> p (n w)", p=ROWS)

    with TileContext(nc) as tc:
        with (
            tc.tile_pool(name="stream", bufs=4) as stream,
            tc.tile_pool(name="escratch", bufs=2) as escratch,
            tc.tile_pool(name="persist", bufs=1) as persist,
            tc.tile_pool(name="small", bufs=1) as small,
            tc.tile_pool(name="psum", bufs=1, space="PSUM") as psum,
            tc.For_i(0, repeat) if repeat > 1 else contextlib.nullcontext(),
        ):
            cellmax = persist.tile([ROWS, NCELLS], dt.float32)
            scol = persist.tile([ROWS, NCHUNK], dt.float32)

            # consts via the ACT HWDGE ring so the SP ring starts streaming at t=0
            c128_sb = small.tile([ROWS, 14], dt.float32)
            nc.scalar.dma_start(out=c128_sb[:], in_=c128.ap())
            c16_sb = small.tile([BPC, 208], dt.float32)
            nc.scalar.dma_start(out=c16_sb[:], in_=c16.ap())
            pec_sb = small.tile([ROWS, 136], dt.float32)
            nc.scalar.dma_start(out=pec_sb[:], in_=pec.ap())
            bsel_t = small.tile([BPC, 2 * ROWS // 2], dt.float32, tag="bsel")
            nc.vector.tensor_copy(bsel_t[:, 0:ROWS], c16_sb[:, 80:208])
            kmask_sb = pec_sb[:, 0:64]
            bsel128_sb = pec_sb[:, 64:80]
            diag8_sb = pec_sb[:, 128:136]
            iota8_sb = c128_sb[:, 0:8]
            eosl_sb = c128_sb[:, 8:9]
            csc_sb = c128_sb[:, 9:10]
            rb393_sb = c128_sb[:, 10:11].bitcast(dt.int32)
            rb8_sb = c128_sb[:, 11:12].bitcast(dt.int32)
            rb16_sb = c128_sb[:, 12:13].bitcast(dt.int32)
            rb8f_sb = c128_sb[:, 13:14]
            iota64_sb = c16_sb[:, 0:64]

            # completed-path static parts
            comb = small.tile([BPC, 2 * K], dt.float32)
            nc.vector.tensor_copy(comb[:, 0:K], c16_sb[:, 64:72])
            clen = small.tile([BPC, 2 * K], dt.float32)
            nc.vector.tensor_copy(clen[:, 0:K], c16_sb[:, 72:80])
            nc.vector.memset(clen[:, K:2 * K], float(L + 1))
            rin = small.tile([BPC, 2 * K], dt.float32)
            nc.vector.reciprocal(rin[:], clen[:])
            eexp = small.tile([ROWS, 1], dt.float32)
            nc.scalar.activation(eexp[:], eosl_sb, AF.Exp)

            # ---------- Phase A: stream logits ----------
            cells_off = 0
            for c, ncell in enumerate(CHUNK_CELLS):
                chunk = ncell * CW
                off = cells_off * CW
                t = stream.tile([ROWS, max(CHUNK_CELLS) * CW], dt.float32, tag="chunk")
                tt = t[:, 0:chunk]
                nc.sync.dma_start(out=tt, in_=lgv[:, off:off + chunk])
                nc.vector.reduce_max(
                    cellmax[:, cells_off:cells_off + ncell],
                    tt.rearrange("p (n w) -> p n w", w=CW),
                    axis=AX.X,
                )
                e = escratch.tile([ROWS, max(CHUNK_CELLS) * CW], dt.float32, tag="exp")
                nc.scalar.activation(e[:, 0:chunk], tt, AF.Exp, accum_out=scol[:, c:c + 1])
                cells_off += ncell

            # ---------- Phase B ----------
            # top-8 cells per row first: DVE is free as soon as the last
            # reduce lands, while lse still waits on the scalar engine.
            c8v = small.tile([ROWS, K], dt.float32)
            nc.vector.max(c8v[:], cellmax[:])
            c8i = small.tile([ROWS, K], dt.uint32)
            nc.vector.max_index(c8i[:], c8v[:], cellmax[:])
            c8ii = small.tile([ROWS, K], dt.int32)
            nc.vector.tensor_copy(c8ii[:], c8i[:])
            rowid = small.tile([ROWS, K], dt.int32)
            nc.vector.tensor_tensor(
                out=rowid[:], in0=c8ii[:], in1=rb393_sb.to_broadcast([ROWS, K]),
                op=OP.add,
            )

            # gather the 8 winning cells per row (one offset per partition/DMA)
            G = persist.tile([ROWS, K * CW], dt.float32)
            for j in range(K):
                nc.gpsimd.indirect_dma_start(
                    out=G[:, j * CW:(j + 1) * CW],
                    out_offset=None,
                    in_=lg.ap(),
                    in_offset=IndirectOffsetOnAxis(ap=rowid[:, j:j + 1], axis=0),
                )

            S = small.tile([ROWS, 1], dt.float32)
            nc.vector.reduce_sum(S[:], scol[:], axis=AX.X)
            S2 = small.tile([ROWS, 1], dt.float32)
            nc.vector.tensor_add(S2[:], S[:], eexp[:])
            lse = small.tile([ROWS, 1], dt.float32)
            nc.scalar.activation(lse[:], S2[:], AF.Ln)
            rc = small.tile([ROWS, 1], dt.float32)
            nc.vector.tensor_sub(rc[:], csc_sb, lse[:])
            eos_adj = small.tile([ROWS, 1], dt.float32)
            nc.vector.tensor_add(eos_adj[:], eosl_sb, rc[:])
            # ---- critical value path: parents -> decoder-state gather ----
            g8v = small.tile([ROWS, K], dt.float32)
            nc.vector.max(g8v[:], G[:])
            badjv = small.tile([ROWS, K], dt.float32)
            nc.vector.tensor_scalar(
                out=badjv[:], in0=g8v[:], scalar1=rc[:, 0:1], scalar2=None,
                op0=OP.add,
            )
            # batch-major transpose via masked rhs + one one-hot matmul (exact):
            # rhsv[q, 8k+j] = badjv[q, j] * (q%8==k); b64v[b, :] = sum_q (q>>3==b) rhsv[q, :]
            rhsv = small.tile([ROWS, K * K], dt.float32)
            nc.vector.tensor_tensor(
                out=rhsv.rearrange("p (a b) -> p a b", a=K),
                in0=kmask_sb.rearrange("p (a b) -> p a b", a=K),
                in1=badjv.unsqueeze(1).to_broadcast([ROWS, K, K]),
                op=OP.mult,
            )
            pb64v = psum.tile([BPC, K * K], dt.float32, tag="pb64v")
            nc.tensor.matmul(out=pb64v[:], lhsT=bsel128_sb, rhs=rhsv[:],
                             start=True, stop=True)
            out40 = small.tile([BPC, 5 * K], dt.float32)
            nc.vector.max(out40[:, 0:K], pb64v[:])       # new_cand_scores
            pos = small.tile([BPC, K], dt.uint32)
            nc.vector.max_index(pos[:], out40[:, 0:K], pb64v[:])
            par_u = small.tile([BPC, K], dt.uint32)
            nc.vector.tensor_scalar(
                out=par_u[:], in0=pos[:], scalar1=3, scalar2=None,
                op0=OP.logical_shift_right,
            )
            paruf = small.tile([BPC, K], dt.float32)
            nc.vector.tensor_copy(paruf[:], par_u[:])
            # replicate parent[b, j] to row-major [128, 1] via K=16 matmul + diag pick
            ppar = psum.tile([ROWS, K], dt.float32, tag="ppar")
            nc.tensor.matmul(out=ppar[:], lhsT=bsel_t[:, 0:ROWS], rhs=paruf[:],
                             start=True, stop=True)
            pdiag = small.tile([ROWS, K], dt.float32)
            nc.vector.tensor_mul(pdiag[:], ppar[:], diag8_sb)
            par128f = small.tile([ROWS, 1], dt.float32)
            nc.vector.reduce_sum(par128f[:], pdiag[:], axis=AX.X)
            prow = small.tile([ROWS, 1], dt.int32)
            nc.vector.tensor_add(prow[:], par128f[:], rb8f_sb)

            # ---------- completed path ----------
            nc.sync.dma_start(out=es_d.ap(), in_=eos_adj[:])
            nc.sync.dma_start(out=comb[:, K:2 * K], in_=es_d.ap().rearrange("(b k) o -> b (k o)", k=K))
            resc = small.tile([BPC, 2 * K], dt.float32)
            nc.vector.tensor_mul(resc[:], comb[:], rin[:])
            cc8 = small.tile([BPC, K], dt.float32)
            nc.vector.max(cc8[:], resc[:])
            cind = small.tile([BPC, K], dt.uint32)
            nc.vector.max_index(cind[:], cc8[:], resc[:])
            cindf = small.tile([BPC, K], dt.float32)
            nc.vector.tensor_copy(cindf[:], cind[:])

            K2 = 2 * K
            m16 = small.tile([BPC, K, K2], dt.float32)
            nc.vector.tensor_tensor(
                out=m16[:],
                in0=iota64_sb[:, 0:K2].unsqueeze(1).to_broadcast([BPC, K, K2]),
                in1=cindf.unsqueeze(2).to_broadcast([BPC, K, K2]),
                op=OP.is_equal,
            )
            p16 = small.tile([BPC, K, K2], dt.float32)
            nc.vector.tensor_tensor(
                out=p16[:], in0=m16[:],
                in1=comb.unsqueeze(1).to_broadcast([BPC, K, K2]),
                op=OP.mult,
            )
            nc.vector.reduce_sum(out40[:, 2 * K:3 * K], p16[:], axis=AX.X)
            p16b = small.tile([BPC, K, K2], dt.float32)
            nc.vector.tensor_tensor(
                out=p16b[:], in0=m16[:],
                in1=clen.unsqueeze(1).to_broadcast([BPC, K, K2]),
                op=OP.mult,
            )
            nc.vector.reduce_sum(out40[:, 3 * K:4 * K], p16b[:], axis=AX.X)

            cindi = small.tile([BPC, K], dt.int32)
            nc.vector.tensor_copy(cindi[:], cind[:])
            nc.sync.dma_start(out=cind_d.ap(), in_=cindi[:])
            crow = small.tile([ROWS, 1], dt.int32)
            nc.sync.dma_start(out=crow[:], in_=cind_d.ap())
            crow2 = small.tile([ROWS, 1], dt.int32)
            nc.vector.tensor_add(crow2[:], crow[:], rb16_sb)
            cseqg = persist.tile([ROWS, MAXL], dt.int32)
            nc.gpsimd.indirect_dma_start(
                out=cseqg[:], out_offset=None, in_=comptab.ap(),
                in_offset=IndirectOffsetOnAxis(ap=crow2[:], axis=0),
            )
            nc.scalar.dma_start(out=cseq_o.ap(), in_=cseqg[:])

            dsg = persist.tile([ROWS, DS], dt.float32)
            H = DS // 4
            for q in range(4):
                nc.gpsimd.indirect_dma_start(
                    out=dsg[:, q * H:(q + 1) * H], out_offset=None, in_=dstate.ap(),
                    in_offset=IndirectOffsetOnAxis(ap=prow[:], axis=0),
                    element_offset=q * H,
                )
                st_eng = nc.sync if q % 2 == 0 else nc.scalar
                st_eng.dma_start(out=dso_o.ap()[:, q * H:(q + 1) * H],
                                 in_=dsg[:, q * H:(q + 1) * H])

            # ---- secondary syms path (vocab ids of the row top-8) ----
            # g8vc copy sits here so MaxIndex can't be hoisted ahead of the
            # critical parent chain by the scheduler.
            g8vc = small.tile([ROWS, K], dt.float32)
            nc.vector.tensor_copy(g8vc[:], g8v[:])
            g8i = small.tile([ROWS, K], dt.uint32)
            nc.vector.max_index(g8i[:], g8vc[:], G[:])
            jstar = small.tile([ROWS, K], dt.uint32)
            nc.vector.tensor_scalar(
                out=jstar[:], in0=g8i[:], scalar1=7, scalar2=None,
                op0=OP.logical_shift_right,
            )
            within = small.tile([ROWS, K], dt.uint32)
            nc.vector.tensor_scalar(
                out=within[:], in0=g8i[:], scalar1=CW - 1, scalar2=None,
                op0=OP.bitwise_and,
            )
            jstar_f = small.tile([ROWS, K], dt.float32)
            nc.vector.tensor_copy(jstar_f[:], jstar[:])
            c8f = small.tile([ROWS, K], dt.float32)
            nc.vector.tensor_copy(c8f[:], c8ii[:])
            withf = small.tile([ROWS, K], dt.float32)
            nc.vector.tensor_copy(withf[:], within[:])

            # cellsel[p, j] = c8f[p, jstar[p, j]]  (batched one-hot gather)
            mask88 = small.tile([ROWS, K, K], dt.float32)
            nc.vector.tensor_tensor(
                out=mask88[:],
                in0=iota8_sb.unsqueeze(1).to_broadcast([ROWS, K, K]),
                in1=jstar_f.unsqueeze(2).to_broadcast([ROWS, K, K]),
                op=OP.is_equal,
            )
            prod88 = small.tile([ROWS, K, K], dt.float32)
            nc.vector.tensor_tensor(
                out=prod88[:],
                in0=mask88[:],
                in1=c8f.unsqueeze(1).to_broadcast([ROWS, K, K]),
                op=OP.mult,
            )
            cellsel = small.tile([ROWS, K], dt.float32)
            nc.vector.reduce_sum(cellsel[:], prod88[:], axis=AX.X)

            voc = small.tile([ROWS, K], dt.float32)
            nc.vector.tensor_scalar(
                out=voc[:], in0=cellsel[:], scalar1=float(CW), scalar2=None,
                op0=OP.mult,
            )
            nc.vector.tensor_add(voc[:], voc[:], withf[:])
            rhss = small.tile([ROWS, K * K], dt.float32)
            nc.vector.tensor_tensor(
                out=rhss.rearrange("p (a b) -> p a b", a=K),
                in0=kmask_sb.rearrange("p (a b) -> p a b", a=K),
                in1=voc.unsqueeze(1).to_broadcast([ROWS, K, K]),
                op=OP.mult,
            )
            pb64s = psum.tile([BPC, K * K], dt.float32, tag="pb64s")
            nc.tensor.matmul(out=pb64s[:], lhsT=bsel128_sb, rhs=rhss[:],
                             start=True, stop=True)
            b64s = small.tile([BPC, K * K], dt.float32)
            nc.vector.tensor_copy(b64s[:], pb64s[:])
            posf = small.tile([BPC, K], dt.float32)
            nc.vector.tensor_copy(posf[:], pos[:])
            nc.vector.tensor_copy(out40[:, K:2 * K], paruf[:])

            # symsel[b, j] = b64s[b, pos[b, j]]
            KK = K * K
            m64 = small.tile([BPC, K, KK], dt.float32)
            nc.vector.tensor_tensor(
                out=m64[:],
                in0=iota64_sb.unsqueeze(1).to_broadcast([BPC, K, KK]),
                in1=posf.unsqueeze(2).to_broadcast([BPC, K, KK]),
                op=OP.is_equal,
            )
            p64 = small.tile([BPC, K, KK], dt.float32)
            nc.vector.tensor_tensor(
                out=p64[:], in0=m64[:],
                in1=b64s.unsqueeze(1).to_broadcast([BPC, K, KK]),
                op=OP.mult,
            )
            nc.vector.reduce_sum(out40[:, 4 * K:5 * K], p64[:], axis=AX.X)
            nc.sync.dma_start(out=out40_o.ap(), in_=out40[:])

    nc.compile()
    return nc


# ---------------------------------------------------------------------------
# host side
# ---------------------------------------------------------------------------

_NC_CACHE = None


def _get_nc():
    global _NC_CACHE
    if _NC_CACHE is None:
        _NC_CACHE = build_nc()
    return _NC_CACHE


def make_in_maps(logits, cand_scores, cand_seqs, completed_scores, completed_seqs,
                 completed_length, decoder_context, decoder_rnn1, decoder_rnn2):
    """Shard + preprocess full inputs into per-core input maps."""
    logits = np.asarray(logits, dtype=_f32)
    cand_scores = np.asarray(cand_scores, dtype=_f32)
    cand_seqs = np.asarray(cand_seqs, dtype=_i32)
    completed_scores = np.asarray(completed_scores, dtype=_f32)
    completed_seqs = np.asarray(completed_seqs, dtype=_i32)
    completed_length = np.asarray(completed_length, dtype=_i32)
    decoder_context = np.asarray(decoder_context, dtype=_f32).reshape(B, K, A)
    decoder_rnn1 = np.asarray(decoder_rnn1, dtype=_f32).reshape(B, K, D)
    decoder_rnn2 = np.asarray(decoder_rnn2, dtype=_f32).reshape(B, K, D)

    p = np.arange(ROWS, dtype=_i32)[:, None]
    c128 = np.empty((ROWS, 14), dtype=_f32)
    c128[:, 0:8] = np.arange(K, dtype=_f32)
    c128[:, 10:11] = (p * NCELLS).astype(_i32).view(_f32)
    c128[:, 11:12] = ((p // K) * K).astype(_i32).view(_f32)
    c128[:, 12:13] = ((p // K) * 2 * K).astype(_i32).view(_f32)
    c128[:, 13:14] = ((p // K) * K).astype(_f32)
    c16 = np.zeros((BPC, 208), dtype=_f32)
    c16[:, 0:64] = np.arange(K * K, dtype=_f32)
    # bsel[q, m] = (q == m >> 3)
    m = np.arange(ROWS)
    c16[:, 80:208] = (np.arange(BPC)[:, None] == (m[None, :] >> 3)).astype(_f32)
    pecv = np.zeros((ROWS, 136), dtype=_f32)
    # kmask[q, 8k+j] = (q % 8 == k); bsel128[q, b] = (q >> 3 == b); diag8[p, j] = (j == p % 8)
    q = np.arange(ROWS)
    kk = np.arange(K * K)
    pecv[:, 0:64] = ((q[:, None] % K) == (kk[None, :] >> 3)).astype(_f32)
    pecv[:, 64:80] = ((q[:, None] >> 3) == np.arange(BPC)[None, :]).astype(_f32)
    pecv[:, 128:136] = (np.arange(K)[None, :] == (q % K)[:, None]).astype(_f32)

    in_maps = []
    for c in range(NCORES):
        b0, b1 = c * BPC, (c + 1) * BPC
        lg2 = logits[b0:b1].reshape(ROWS, V)
        cc128 = c128.copy()
        cc128[:, 8:9] = lg2[:, EOS:EOS + 1]
        cc128[:, 9:10] = cand_scores[b0:b1].reshape(ROWS, 1)
        cc16 = c16.copy()
        cc16[:, 64:72] = completed_scores[b0:b1]
        cc16[:, 72:80] = completed_length[b0:b1].astype(_f32)

        lgp = np.full((ROWS, VP), NEG, dtype=_f32)
        lgp[:, :V] = lg2
        lgp[:, EOS] = NEG

        # completed table rows (b*16 + c): c<8 completed_seqs, c>=8 cand_seqs+EOS
        ctab = np.full((BPC, 2 * K, MAXL), EOS, dtype=_i32)
        ctab[:, :K, :] = completed_seqs[b0:b1]
        ctab[:, K:, :L] = cand_seqs[b0:b1]

        ds = np.empty((ROWS, DS), dtype=_f32)
        ds[:, 0:A] = decoder_context[b0:b1].reshape(ROWS, A)
        ds[:, A:A + D] = decoder_rnn1[b0:b1].reshape(ROWS, D)
        ds[:, A + D:A + 2 * D] = decoder_rnn2[b0:b1].reshape(ROWS, D)
        ds[:, A + 2 * D:DS] = cand_seqs[b0:b1].reshape(ROWS, L).view(_f32)

        in_maps.append({
            "lg": lgp.reshape(ROWS * NCELLS, CW),
            "c128": cc128,
            "c16": cc16,
            "comptab": ctab.reshape(BPC * 2 * K, MAXL),
            "dstate": ds,
            "pec": pecv,
        })
    return in_maps


def assemble_outputs(results):
    """Concatenate per-core output dicts into the reference's 9-tuple."""
    cat = lambda name: np.concatenate([np.asarray(r[name]) for r in results], axis=0)
    out40 = cat("out40").reshape(B, 5 * K)
    new_cand_scores = np.ascontiguousarray(out40[:, 0:K])
    parents = np.rint(out40[:, K:2 * K]).astype(_i32)
    comp_scores = np.ascontiguousarray(out40[:, 2 * K:3 * K])
    comp_len = np.rint(out40[:, 3 * K:4 * K]).astype(_i32)
    syms = np.rint(out40[:, 4 * K:5 * K]).astype(_i32)
    comp_seqs = cat("cseqo").reshape(B, K, MAXL).astype(_i32)
    dso = cat("dso").reshape(B * K, DS)
    ctx = np.ascontiguousarray(dso[:, 0:A])
    r1 = np.ascontiguousarray(dso[:, A:A + D])
    r2 = np.ascontiguousarray(dso[:, A + D:A + 2 * D])
    seqs = np.ascontiguousarray(dso[:, A + 2 * D:DS]).view(_i32).reshape(B, K, L)
    new_seqs = np.concatenate([seqs, syms.reshape(B, K, 1)], axis=2)
    return (new_cand_scores, new_seqs, parents, comp_scores, comp_seqs,
            comp_len, ctx, r1, r2)


def kernel(**inputs):
    from concourse.bass_utils import run_bass_kernel_spmd
    nc = _get_nc()
    in_maps = make_in_maps(**inputs)
    res = run_bass_kernel_spmd(nc, in_maps, list(range(NCORES)))
    return assemble_outputs(res.results)


# revision 35
# speedup vs baseline: 1.2191x; 1.0102x over previous
"""Batch beam-search step kernel for Trainium2 (8 NeuronCores, batch-sharded).

Reference semantics (per batch b, beams K=8, vocab V=50257):
  logprobs = log_softmax(logits); new = logprobs + cand_scores[:, None]
  eos = new[:, :, EOS]; completed top-k over concat(completed, eos)/len
  uncompleted top-8 over new with EOS masked, gather seqs + decoder state.

Sharding: batch dim 128 -> 16 batches per core (128 (b,k) rows = 128
SBUF partitions per core).

Device algorithm per core:
  Phase A (streaming, memory-bound): logits padded to 393*128 columns,
  streamed in tapered chunks; per chunk a 3D reduce_max produces per-cell
  (128-wide) maxes while Exp(+accum) computes the softmax denominator in
  parallel on the scalar engine.
  Phase B: logsumexp from chunk sums; top-8 cells per row via max/max_index
  over the 393 cell maxes; 8 indirect DMAs gather the winning cells;
  max/max_index over the gathered 1K recovers the row-global top-8 values
  and vocab ids.  Per-batch top-8 over the 8x8 row candidates via a DRAM
  bounce to batch-major layout; beam reorder of seqs + decoder state via
  one indirect row gather by parent over a host-packed [ctx|r1|r2|seq]
  table.
"""

import numpy as np

B, K, V = 128, 8, 50257
L, MAXL = 64, 128
A, D = 512, 1024
EOS = 2
NEG = -1.0e9

NCORES = 8
BPC = B // NCORES          # 16 batches per core
ROWS = BPC * K             # 128 rows = partitions
CW = 128                   # cell width
NCELLS = 393               # ceil(V / CW)
VP = NCELLS * CW           # 50304 padded vocab
# cells per streaming chunk (tapered tail so the last reduce+exp are short)
CHUNK_CELLS = [45, 45, 45, 45, 45, 45, 45, 36, 21, 12, 6, 3]
assert sum(CHUNK_CELLS) == NCELLS
NCHUNK = len(CHUNK_CELLS)
DS = A + 2 * D + L         # packed decoder-state row: ctx|r1|r2|seq = 2624

_f32 = np.float32
_i32 = np.int32


def build_nc(repeat=1):
    import contextlib
    import concourse.bacc as bacc
    import concourse.mybir as mybir
    from concourse.bass import IndirectOffsetOnAxis
    from concourse.tile import TileContext

    dt = mybir.dt
    AX = mybir.AxisListType
    OP = mybir.AluOpType
    AF = mybir.ActivationFunctionType

    nc = bacc.Bacc("TRN2", target_bir_lowering=False, num_devices=NCORES)

    # ---- inputs ----
    lg = nc.declare_dram_parameter("lg", [ROWS * NCELLS, CW], dt.float32, isOutput=False)
    # c128: iota8(0:8)|eosl(8)|csc(9)|rb393(10,i32)|rb8(11,i32)|rb16(12,i32)|rb8f(13)
    c128 = nc.declare_dram_parameter("c128", [ROWS, 14], dt.float32, isOutput=False)
    # c16: iota64(0:64) | compsc(64:72) | complenf(72:80) | bsel(80:208)
    c16 = nc.declare_dram_parameter("c16", [BPC, 208], dt.float32, isOutput=False)
    # pec: lgsel selectors (0:128; selector k at cols 16k..16k+16) | diag8 (128:136)
    pec = nc.declare_dram_parameter("pec", [ROWS, 136], dt.float32, isOutput=False)
    comptab = nc.declare_dram_parameter("comptab", [BPC * 2 * K, MAXL], dt.int32, isOutput=False)
    dstate = nc.declare_dram_parameter("dstate", [ROWS, DS], dt.float32, isOutput=False)

    # ---- outputs ----
    # out40 packs [ncs | parents | comp_scores | comp_len | syms] per batch (f32).
    out40_o = nc.declare_dram_parameter("out40", [BPC, 5 * K], dt.float32, isOutput=True)
    cseq_o = nc.declare_dram_parameter("cseqo", [ROWS, MAXL], dt.int32, isOutput=True)
    dso_o = nc.declare_dram_parameter("dso", [ROWS, DS], dt.float32, isOutput=True)

    # ---- DRAM bounce scratch ----
    par_d = nc.dram_tensor("par_d", [BPC, K], dt.int32)
    es_d = nc.dram_tensor("es_d", [ROWS, 1], dt.float32)
    cind_d = nc.dram_tensor("cind_d", [BPC, K], dt.int32)

    lgv = lg.ap().rearrange("(p n) w -> p (n w)", p=ROWS)

    with TileContext(nc) as tc:
        with (
            tc.tile_pool(name="stream", bufs=4) as stream,
            tc.tile_pool(name="escratch", bufs=2) as escratch,
            tc.tile_pool(name="persist", bufs=1) as persist,
            tc.tile_pool(name="small", bufs=1) as small,
            tc.tile_pool(name="psum", bufs=1, space="PSUM") as psum,
            tc.For_i(0, repeat) if repeat > 1 else contextlib.nullcontext(),
        ):
            cellmax = persist.tile([ROWS, NCELLS], dt.float32)
            scol = persist.tile([ROWS, NCHUNK], dt.float32)

            # consts via the ACT HWDGE ring so the SP ring starts streaming at t=0
            c128_sb = small.tile([ROWS, 14], dt.float32)
            nc.scalar.dma_start(out=c128_sb[:], in_=c128.ap())
            c16_sb = small.tile([BPC, 208], dt.float32)
            nc.scalar.dma_start(out=c16_sb[:], in_=c16.ap())
            pec_sb = small.tile([ROWS, 136], dt.float32)
            nc.scalar.dma_start(out=pec_sb[:], in_=pec.ap())
            bsel_t = small.tile([BPC, 2 * ROWS // 2], dt.float32, tag="bsel")
            nc.vector.tensor_copy(bsel_t[:, 0:ROWS], c16_sb[:, 80:208])
            kmask_sb = pec_sb[:, 0:64]
            bsel128_sb = pec_sb[:, 64:80]
            diag8_sb = pec_sb[:, 128:136]
            iota8_sb = c128_sb[:, 0:8]
            eosl_sb = c128_sb[:, 8:9]
            csc_sb = c128_sb[:, 9:10]
            rb393_sb = c128_sb[:, 10:11].bitcast(dt.int32)
            rb8_sb = c128_sb[:, 11:12].bitcast(dt.int32)
            rb16_sb = c128_sb[:, 12:13].bitcast(dt.int32)
            rb8f_sb = c128_sb[:, 13:14]
            iota64_sb = c16_sb[:, 0:64]

            # completed-path static parts
            comb = small.tile([BPC, 2 * K], dt.float32)
            nc.vector.tensor_copy(comb[:, 0:K], c16_sb[:, 64:72])
            clen = small.tile([BPC, 2 * K], dt.float32)
            nc.vector.tensor_copy(clen[:, 0:K], c16_sb[:, 72:80])
            nc.vector.memset(clen[:, K:2 * K], float(L + 1))
            rin = small.tile([BPC, 2 * K], dt.float32)
            nc.vector.reciprocal(rin[:], clen[:])
            eexp = small.tile([ROWS, 1], dt.float32)
            nc.scalar.activation(eexp[:], eosl_sb, AF.Exp)

            # ---------- Phase A: stream logits ----------
            cells_off = 0
            for c, ncell in enumerate(CHUNK_CELLS):
                chunk = ncell * CW
                off = cells_off * CW
                t = stream.tile([ROWS, max(CHUNK_CELLS) * CW], dt.float32, tag="chunk")
                tt = t[:, 0:chunk]
                nc.sync.dma_start(out=tt, in_=lgv[:, off:off + chunk])
                nc.vector.reduce_max(
                    cellmax[:, cells_off:cells_off + ncell],
                    tt.rearrange("p (n w) -> p n w", w=CW),
                    axis=AX.X,
                )
                e = escratch.tile([ROWS, max(CHUNK_CELLS) * CW], dt.float32, tag="exp")
                nc.scalar.activation(e[:, 0:chunk], tt, AF.Exp, accum_out=scol[:, c:c + 1])
                cells_off += ncell

            # ---------- Phase B ----------
            # top-8 cells per row first: DVE is free as soon as the last
            # reduce lands, while lse still waits on the scalar engine.
            c8v = small.tile([ROWS, K], dt.float32)
            nc.vector.max(c8v[:], cellmax[:])
            c8i = small.tile([ROWS, K], dt.uint32)
            nc.vector.max_index(c8i[:], c8v[:], cellmax[:])
            c8ii = small.tile([ROWS, K], dt.int32)
            nc.vector.tensor_copy(c8ii[:], c8i[:])
            rowid = small.tile([ROWS, K], dt.int32)
            nc.vector.tensor_tensor(
                out=rowid[:], in0=c8ii[:], in1=rb393_sb.to_broadcast([ROWS, K]),
                op=OP.add,
            )

            # gather the 8 winning cells per row (one offset per partition/DMA)
            G = persist.tile([ROWS, K * CW], dt.float32)
            for j in range(K):
                nc.gpsimd.indirect_dma_start(
                    out=G[:, j * CW:(j + 1) * CW],
                    out_offset=None,
                    in_=lg.ap(),
                    in_offset=IndirectOffsetOnAxis(ap=rowid[:, j:j + 1], axis=0),
                )

            S = small.tile([ROWS, 1], dt.float32)
            nc.vector.reduce_sum(S[:], scol[:], axis=AX.X)
            S2 = small.tile([ROWS, 1], dt.float32)
            nc.vector.tensor_add(S2[:], S[:], eexp[:])
            lse = small.tile([ROWS, 1], dt.float32)
            nc.scalar.activation(lse[:], S2[:], AF.Ln)
            rc = small.tile([ROWS, 1], dt.float32)
            nc.vector.tensor_sub(rc[:], csc_sb, lse[:])
            eos_adj = small.tile([ROWS, 1], dt.float32)
            nc.vector.tensor_add(eos_adj[:], eosl_sb, rc[:])
            # ---- critical value path: parents -> decoder-state gather ----
            # half-maxes overlap the gather stream; top-8 of the two top-8s
            # equals the global top-8
            gh = small.tile([ROWS, 2 * K], dt.float32)
            nc.vector.max(gh[:, 0:K], G[:, 0:K * CW // 2])
            nc.vector.max(gh[:, K:2 * K], G[:, K * CW // 2:K * CW])
            g8v = small.tile([ROWS, K], dt.float32)
            nc.vector.max(g8v[:], gh[:])
            badjv = small.tile([ROWS, K], dt.float32)
            nc.vector.tensor_scalar(
                out=badjv[:], in0=g8v[:], scalar1=rc[:, 0:1], scalar2=None,
                op0=OP.add,
            )
            # batch-major transpose via masked rhs + one one-hot matmul (exact):
            # rhsv[q, 8k+j] = badjv[q, j] * (q%8==k); b64v[b, :] = sum_q (q>>3==b) rhsv[q, :]
            rhsv = small.tile([ROWS, K * K], dt.float32)
            nc.vector.tensor_tensor(
                out=rhsv.rearrange("p (a b) -> p a b", a=K),
                in0=kmask_sb.rearrange("p (a b) -> p a b", a=K),
                in1=badjv.unsqueeze(1).to_broadcast([ROWS, K, K]),
                op=OP.mult,
            )
            pb64v = psum.tile([BPC, K * K], dt.float32, tag="pb64v")
            nc.tensor.matmul(out=pb64v[:], lhsT=bsel128_sb, rhs=rhsv[:],
                             start=True, stop=True)
            out40 = small.tile([BPC, 5 * K], dt.float32)
            nc.vector.max(out40[:, 0:K], pb64v[:])       # new_cand_scores
            pos = small.tile([BPC, K], dt.uint32)
            nc.vector.max_index(pos[:], out40[:, 0:K], pb64v[:])
            par_u = small.tile([BPC, K], dt.uint32)
            nc.vector.tensor_scalar(
                out=par_u[:], in0=pos[:], scalar1=3, scalar2=None,
                op0=OP.logical_shift_right,
            )
            paruf = small.tile([BPC, K], dt.float32)
            nc.vector.tensor_copy(paruf[:], par_u[:])
            # replicate parent[b, j] to row-major [128, 1] via K=16 matmul + diag pick
            ppar = psum.tile([ROWS, K], dt.float32, tag="ppar")
            nc.tensor.matmul(out=ppar[:], lhsT=bsel_t[:, 0:ROWS], rhs=paruf[:],
                             start=True, stop=True)
            pdiag = small.tile([ROWS, K], dt.float32)
            nc.vector.tensor_mul(pdiag[:], ppar[:], diag8_sb)
            par128f = small.tile([ROWS, 1], dt.float32)
            nc.vector.reduce_sum(par128f[:], pdiag[:], axis=AX.X)
            prow = small.tile([ROWS, 1], dt.int32)
            nc.vector.tensor_add(prow[:], par128f[:], rb8f_sb)

            # ---------- completed path ----------
            nc.sync.dma_start(out=es_d.ap(), in_=eos_adj[:])
            nc.sync.dma_start(out=comb[:, K:2 * K], in_=es_d.ap().rearrange("(b k) o -> b (k o)", k=K))
            resc = small.tile([BPC, 2 * K], dt.float32)
            nc.vector.tensor_mul(resc[:], comb[:], rin[:])
            cc8 = small.tile([BPC, K], dt.float32)
            nc.vector.max(cc8[:], resc[:])
            cind = small.tile([BPC, K], dt.uint32)
            nc.vector.max_index(cind[:], cc8[:], resc[:])
            cindf = small.tile([BPC, K], dt.float32)
            nc.vector.tensor_copy(cindf[:], cind[:])

            K2 = 2 * K
            m16 = small.tile([BPC, K, K2], dt.float32)
            nc.vector.tensor_tensor(
                out=m16[:],
                in0=iota64_sb[:, 0:K2].unsqueeze(1).to_broadcast([BPC, K, K2]),
                in1=cindf.unsqueeze(2).to_broadcast([BPC, K, K2]),
                op=OP.is_equal,
            )
            p16 = small.tile([BPC, K, K2], dt.float32)
            nc.vector.tensor_tensor(
                out=p16[:], in0=m16[:],
                in1=comb.unsqueeze(1).to_broadcast([BPC, K, K2]),
                op=OP.mult,
            )
            nc.vector.reduce_sum(out40[:, 2 * K:3 * K], p16[:], axis=AX.X)
            p16b = small.tile([BPC, K, K2], dt.float32)
            nc.vector.tensor_tensor(
                out=p16b[:], in0=m16[:],
                in1=clen.unsqueeze(1).to_broadcast([BPC, K, K2]),
                op=OP.mult,
            )
            nc.vector.reduce_sum(out40[:, 3 * K:4 * K], p16b[:], axis=AX.X)

            cindi = small.tile([BPC, K], dt.int32)
            nc.vector.tensor_copy(cindi[:], cind[:])
            nc.sync.dma_start(out=cind_d.ap(), in_=cindi[:])
            crow = small.tile([ROWS, 1], dt.int32)
            nc.sync.dma_start(out=crow[:], in_=cind_d.ap())
            crow2 = small.tile([ROWS, 1], dt.int32)
            nc.vector.tensor_add(crow2[:], crow[:], rb16_sb)
            cseqg = persist.tile([ROWS, MAXL], dt.int32)
            nc.gpsimd.indirect_dma_start(
                out=cseqg[:], out_offset=None, in_=comptab.ap(),
                in_offset=IndirectOffsetOnAxis(ap=crow2[:], axis=0),
            )
            nc.scalar.dma_start(out=cseq_o.ap(), in_=cseqg[:])

            dsg = persist.tile([ROWS, DS], dt.float32)
            bounds = [0, 768, 1536, 2304, DS]
            for q in range(4):
                lo, hi = bounds[q], bounds[q + 1]
                nc.gpsimd.indirect_dma_start(
                    out=dsg[:, lo:hi], out_offset=None, in_=dstate.ap(),
                    in_offset=IndirectOffsetOnAxis(ap=prow[:], axis=0),
                    element_offset=lo,
                )
                st_eng = nc.sync if q % 2 == 0 else nc.scalar
                st_eng.dma_start(out=dso_o.ap()[:, lo:hi], in_=dsg[:, lo:hi])

            # ---- secondary syms path (vocab ids of the row top-8) ----
            # g8vc copy sits here so MaxIndex can't be hoisted ahead of the
            # critical parent chain by the scheduler.
            g8vc = small.tile([ROWS, K], dt.float32)
            nc.vector.tensor_copy(g8vc[:], g8v[:])
            g8i = small.tile([ROWS, K], dt.uint32)
            nc.vector.max_index(g8i[:], g8vc[:], G[:])
            jstar = small.tile([ROWS, K], dt.uint32)
            nc.vector.tensor_scalar(
                out=jstar[:], in0=g8i[:], scalar1=7, scalar2=None,
                op0=OP.logical_shift_right,
            )
            within = small.tile([ROWS, K], dt.uint32)
            nc.vector.tensor_scalar(
                out=within[:], in0=g8i[:], scalar1=CW - 1, scalar2=None,
                op0=OP.bitwise_and,
            )
            jstar_f = small.tile([ROWS, K], dt.float32)
            nc.vector.tensor_copy(jstar_f[:], jstar[:])
            c8f = small.tile([ROWS, K], dt.float32)
            nc.vector.tensor_copy(c8f[:], c8ii[:])
            withf = small.tile([ROWS, K], dt.float32)
            nc.vector.tensor_copy(withf[:], within[:])

            # cellsel[p, j] = c8f[p, jstar[p, j]]  (batched one-hot gather)
            mask88 = small.tile([ROWS, K, K], dt.float32)
            nc.vector.tensor_tensor(
                out=mask88[:],
                in0=iota8_sb.unsqueeze(1).to_broadcast([ROWS, K, K]),
                in1=jstar_f.unsqueeze(2).to_broadcast([ROWS, K, K]),
                op=OP.is_equal,
            )
            prod88 = small.tile([ROWS, K, K], dt.float32)
            nc.vector.tensor_tensor(
                out=prod88[:],
                in0=mask88[:],
                in1=c8f.unsqueeze(1).to_broadcast([ROWS, K, K]),
                op=OP.mult,
            )
            cellsel = small.tile([ROWS, K], dt.float32)
            nc.vector.reduce_sum(cellsel[:], prod88[:], axis=AX.X)

            voc = small.tile([ROWS, K], dt.float32)
            nc.vector.tensor_scalar(
                out=voc[:], in0=cellsel[:], scalar1=float(CW), scalar2=None,
                op0=OP.mult,
            )
            nc.vector.tensor_add(voc[:], voc[:], withf[:])
            rhss = small.tile([ROWS, K * K], dt.float32)
            nc.vector.tensor_tensor(
                out=rhss.rearrange("p (a b) -> p a b", a=K),
                in0=kmask_sb.rearrange("p (a b) -> p a b", a=K),
                in1=voc.unsqueeze(1).to_broadcast([ROWS, K, K]),
                op=OP.mult,
            )
            pb64s = psum.tile([BPC, K * K], dt.float32, tag="pb64s")
            nc.tensor.matmul(out=pb64s[:], lhsT=bsel128_sb, rhs=rhss[:],
                             start=True, stop=True)
            b64s = small.tile([BPC, K * K], dt.float32)
            nc.vector.tensor_copy(b64s[:], pb64s[:])
            posf = small.tile([BPC, K], dt.float32)
            nc.vector.tensor_copy(posf[:], pos[:])
            nc.vector.tensor_copy(out40[:, K:2 * K], paruf[:])

            # symsel[b, j] = b64s[b, pos[b, j]]
            KK = K * K
            m64 = small.tile([BPC, K, KK], dt.float32)
            nc.vector.tensor_tensor(
                out=m64[:],
                in0=iota64_sb.unsqueeze(1).to_broadcast([BPC, K, KK]),
                in1=posf.unsqueeze(2).to_broadcast([BPC, K, KK]),
                op=OP.is_equal,
            )
            p64 = small.tile([BPC, K, KK], dt.float32)
            nc.vector.tensor_tensor(
                out=p64[:], in0=m64[:],
                in1=b64s.unsqueeze(1).to_broadcast([BPC, K, KK]),
                op=OP.mult,
            )
            nc.vector.reduce_sum(out40[:, 4 * K:5 * K], p64[:], axis=AX.X)
            nc.sync.dma_start(out=out40_o.ap(), in_=out40[:])

    nc.compile()
    return nc


# ---------------------------------------------------------------------------
# host side
# ---------------------------------------------------------------------------

_NC_CACHE = None


def _get_nc():
    global _NC_CACHE
    if _NC_CACHE is None:
        _NC_CACHE = build_nc()
    return _NC_CACHE


def make_in_maps(logits, cand_scores, cand_seqs, completed_scores, completed_seqs,
                 completed_length, decoder_context, decoder_rnn1, decoder_rnn2):
    """Shard + preprocess full inputs into per-core input maps."""
    logits = np.asarray(logits, dtype=_f32)
    cand_scores = np.asarray(cand_scores, dtype=_f32)
    cand_seqs = np.asarray(cand_seqs, dtype=_i32)
    completed_scores = np.asarray(completed_scores, dtype=_f32)
    completed_seqs = np.asarray(completed_seqs, dtype=_i32)
    completed_length = np.asarray(completed_length, dtype=_i32)
    decoder_context = np.asarray(decoder_context, dtype=_f32).reshape(B, K, A)
    decoder_rnn1 = np.asarray(decoder_rnn1, dtype=_f32).reshape(B, K, D)
    decoder_rnn2 = np.asarray(decoder_rnn2, dtype=_f32).reshape(B, K, D)

    p = np.arange(ROWS, dtype=_i32)[:, None]
    c128 = np.empty((ROWS, 14), dtype=_f32)
    c128[:, 0:8] = np.arange(K, dtype=_f32)
    c128[:, 10:11] = (p * NCELLS).astype(_i32).view(_f32)
    c128[:, 11:12] = ((p // K) * K).astype(_i32).view(_f32)
    c128[:, 12:13] = ((p // K) * 2 * K).astype(_i32).view(_f32)
    c128[:, 13:14] = ((p // K) * K).astype(_f32)
    c16 = np.zeros((BPC, 208), dtype=_f32)
    c16[:, 0:64] = np.arange(K * K, dtype=_f32)
    # bsel[q, m] = (q == m >> 3)
    m = np.arange(ROWS)
    c16[:, 80:208] = (np.arange(BPC)[:, None] == (m[None, :] >> 3)).astype(_f32)
    pecv = np.zeros((ROWS, 136), dtype=_f32)
    # kmask[q, 8k+j] = (q % 8 == k); bsel128[q, b] = (q >> 3 == b); diag8[p, j] = (j == p % 8)
    q = np.arange(ROWS)
    kk = np.arange(K * K)
    pecv[:, 0:64] = ((q[:, None] % K) == (kk[None, :] >> 3)).astype(_f32)
    pecv[:, 64:80] = ((q[:, None] >> 3) == np.arange(BPC)[None, :]).astype(_f32)
    pecv[:, 128:136] = (np.arange(K)[None, :] == (q % K)[:, None]).astype(_f32)

    in_maps = []
    for c in range(NCORES):
        b0, b1 = c * BPC, (c + 1) * BPC
        lg2 = logits[b0:b1].reshape(ROWS, V)
        cc128 = c128.copy()
        cc128[:, 8:9] = lg2[:, EOS:EOS + 1]
        cc128[:, 9:10] = cand_scores[b0:b1].reshape(ROWS, 1)
        cc16 = c16.copy()
        cc16[:, 64:72] = completed_scores[b0:b1]
        cc16[:, 72:80] = completed_length[b0:b1].astype(_f32)

        lgp = np.full((ROWS, VP), NEG, dtype=_f32)
        lgp[:, :V] = lg2
        lgp[:, EOS] = NEG

        # completed table rows (b*16 + c): c<8 completed_seqs, c>=8 cand_seqs+EOS
        ctab = np.full((BPC, 2 * K, MAXL), EOS, dtype=_i32)
        ctab[:, :K, :] = completed_seqs[b0:b1]
        ctab[:, K:, :L] = cand_seqs[b0:b1]

        ds = np.empty((ROWS, DS), dtype=_f32)
        ds[:, 0:A] = decoder_context[b0:b1].reshape(ROWS, A)
        ds[:, A:A + D] = decoder_rnn1[b0:b1].reshape(ROWS, D)
        ds[:, A + D:A + 2 * D] = decoder_rnn2[b0:b1].reshape(ROWS, D)
        ds[:, A + 2 * D:DS] = cand_seqs[b0:b1].reshape(ROWS, L).view(_f32)

        in_maps.append({
            "lg": lgp.reshape(ROWS * NCELLS, CW),
            "c128": cc128,
            "c16": cc16,
            "comptab": ctab.reshape(BPC * 2 * K, MAXL),
            "dstate": ds,
            "pec": pecv,
        })
    return in_maps


def assemble_outputs(results):
    """Concatenate per-core output dicts into the reference's 9-tuple."""
    cat = lambda name: np.concatenate([np.asarray(r[name]) for r in results], axis=0)
    out40 = cat("out40").reshape(B, 5 * K)
    new_cand_scores = np.ascontiguousarray(out40[:, 0:K])
    parents = np.rint(out40[:, K:2 * K]).astype(_i32)
    comp_scores = np.ascontiguousarray(out40[:, 2 * K:3 * K])
    comp_len = np.rint(out40[:, 3 * K:4 * K]).astype(_i32)
    syms = np.rint(out40[:, 4 * K:5 * K]).astype(_i32)
    comp_seqs = cat("cseqo").reshape(B, K, MAXL).astype(_i32)
    dso = cat("dso").reshape(B * K, DS)
    ctx = np.ascontiguousarray(dso[:, 0:A])
    r1 = np.ascontiguousarray(dso[:, A:A + D])
    r2 = np.ascontiguousarray(dso[:, A + D:A + 2 * D])
    seqs = np.ascontiguousarray(dso[:, A + 2 * D:DS]).view(_i32).reshape(B, K, L)
    new_seqs = np.concatenate([seqs, syms.reshape(B, K, 1)], axis=2)
    return (new_cand_scores, new_seqs, parents, comp_scores, comp_seqs,
            comp_len, ctx, r1, r2)


def kernel(**inputs):
    from concourse.bass_utils import run_bass_kernel_spmd
    nc = _get_nc()
    in_maps = make_in_maps(**inputs)
    res = run_bass_kernel_spmd(nc, in_maps, list(range(NCORES)))
    return assemble_outputs(res.results)
